# revision 29
# baseline (speedup 1.0000x reference)
"""Trainium2 Bass kernel for Llama-style GQA attention (B=1, S=2048, D=4096,
H=32 q-heads, KVH=8 kv-heads, HD=128, rope theta=1e6, causal, all-ones
attention mask).

Sharding: tensor-parallel over heads across 8 NeuronCores. Core c owns q-heads
[4c, 4c+4) and kv-head c (wq/wkv column shards, fp16). Attention context is
AllGathered feature-sharded (fp16, per 512-token q-chunk) and the output
projection is column-parallel; the host concatenates the 8 fp32 column shards.

Device pipeline per core (fp16 matmuls, fp32 PSUM accumulation):
  1. QKV projections from pre-transposed fp16 X^T: stationary X^T tiles,
     moving weight chunks, double-buffered PSUM; wq/wkv chunk DMAs
     just-in-time; cos/sin tables are per-head-shared [128, NT, HD] with
     broadcast APs (1MB instead of 9MB of table traffic); wo/dmask loads are
     deferred behind the QKV stream on the gpsimd queue.
  2. RoPE in token-major layout (3 DVE muls + 1 add per tile), PE transposes
     to head-dim-major qt/kt interleaved into the QKV groups one group late
     so they never stall on the DVE rope chain.
  3. Attention per (head, 512-token q-chunk), software-pipelined DEPTH=3:
     S^T = KT.T @ QT on PE; exp on ScalarE with constant bias (-8ln2 folds a
     2^-8 scale into the fp16 exp sums; mask is all-ones by problem spec);
     causal masking folded into the scores on PE (identity matmul adds -6e4
     below the diagonal, exp underflows to exact 0); everything on diagonal
     tiles (S matmul, bias, exp, PV, denominator adds) is column-trimmed to
     the causally visible range. Softmax denominators accumulate incrementally on
     DVE; reciprocal+broadcast runs as recip(DVE) -> ones-row matmul
     broadcast (PE) -> fp16 copy (ACT) -> normalize (DVE) — no GpSimd
     PartitionBroadcast (3-5us each) on the critical path. The den/bc stages
     of head h are deferred into head h+1's k-loop to keep PE fed.
  4. Chunked AllGather of fp16 ctx^T shards (gpsimd SWDGE + doorbell).
  5. Output projection: 4 double-buffered accumulators, cc chunk DMAs
     prefetched through a 16-deep pool.
"""

import math

import numpy as np

import concourse.bass as bass
import concourse.bacc as bacc_mod
import concourse.mybir as mybir
import concourse.tile as tile
from concourse.bass_utils import run_bass_kernel_spmd
from concourse.masks import make_identity

S = 2048
D = 4096
H = 32
KVH = 8
HD = 128
NC = 8
HPC = H // NC          # 4 q heads per core
QF = HPC * HD          # 512 q features per core
NT = S // 128          # 16 token tiles
KC = D // 128          # 32 contraction chunks
QCH = S // 512         # 4 q-chunks of 512
THETA = 1e6
SCALE = 1.0 / math.sqrt(HD)
EXP_BIAS = -8.0 * math.log(2.0)   # 2^-8 pre-scale; cancels in normalization
F16 = mybir.dt.float16
F32 = mybir.dt.float32

_CACHE = {}


def build_program():
    nc = bacc_mod.Bacc(None, num_devices=NC)

    xt_d = nc.dram_tensor("xt", [D, S], F16, kind="ExternalInput")
    wq_d = nc.dram_tensor("wq", [D, QF], F16, kind="ExternalInput")
    wkv_d = nc.dram_tensor("wkv", [D, 256], F16, kind="ExternalInput")
    wo_d = nc.dram_tensor("wo", [D, QF], F16, kind="ExternalInput")
    cosk_d = nc.dram_tensor("cosk", [128, NT * HD], F16, kind="ExternalInput")
    sink_d = nc.dram_tensor("sink", [128, NT * HD], F16, kind="ExternalInput")
    dbias_d = nc.dram_tensor("dbias", [128, QCH, 512], F16, kind="ExternalInput")
    out_d = nc.dram_tensor("out", [S, QF], F32, kind="ExternalOutput")

    cc_in = [nc.dram_tensor(f"cc_in{i}", [QF, 512], F16) for i in range(QCH)]
    cc_out = [nc.dram_tensor(f"cc_out{i}", [D, 512], F16, addr_space="Shared")
              for i in range(QCH)]

    with tile.TileContext(nc) as tc:
        with (
            tc.tile_pool(name="const", bufs=1) as const,
            tc.tile_pool(name="wqkv", bufs=1) as wqkv_pool,
            tc.tile_pool(name="wo_pool", bufs=1) as wo_pool,
            tc.tile_pool(name="attn_sb", bufs=1) as attn_sb,
        ):
            # rope tables first on the gpsimd queue: needed ~10us in
            cosk = const.tile([128, NT, HD], F16, tag="cosk")
            sink = const.tile([128, NT, HD], F16, tag="sink")
            ident = const.tile([128, 128], F16, tag="ident")
            ones_col = const.tile([128, 1], F16, tag="ones_col")
            ones_row = const.tile([1, 128], F16, tag="ones_row")
            dbias = const.tile([128, QCH, 512], F16, tag="dbias")
            ebias = const.tile([128, 1], F32, tag="ebias")
            nc.gpsimd.memset(ebias[:], EXP_BIAS)
            nc.gpsimd.dma_start(cosk[:].rearrange("p t f -> p (t f)"), cosk_d[:])
            nc.gpsimd.dma_start(sink[:].rearrange("p t f -> p (t f)"), sink_d[:])
            make_identity(nc, ident[:])
            nc.gpsimd.memset(ones_col[:], 1.0)
            nc.gpsimd.memset(ones_row[:], 1.0)
            # dbias is first needed by attention (~250us in); keep it behind
            # the rope tables on the gpsimd queue
            nc.gpsimd.dma_start(dbias[:], dbias_d[:])

            wq = wqkv_pool.tile([128, KC, QF], F16, tag="wq")
            wkv = wqkv_pool.tile([128, KC, 256], F16, tag="wkv")
            wo = wo_pool.tile([128, KC, QF], F16, tag="wo")

            # attention operands (persistent through phases 2-4)
            qt = attn_sb.tile([128, HPC, S], F16, tag="qt")    # [hd, head, tok]
            kt = attn_sb.tile([128, S], F16, tag="kt")         # [hd, tok]
            vv = attn_sb.tile([128, NT, HD], F16, tag="v")     # [tok%128, tile, hd]
            ctxc = attn_sb.tile([128, HPC, S], F16, tag="ctxc")

            # ---------- phase 1: QKV + RoPE + interleaved transposes ----------
            with (
                tc.tile_pool(name="stage", bufs=4) as stage,
                tc.tile_pool(name="xtp", bufs=24) as xtp,
                tc.tile_pool(name="rope_tmp", bufs=4) as ropep,
                tc.tile_pool(name="qkv_ps", bufs=2, space="PSUM") as qkv_ps,
                tc.tile_pool(name="tr_ps", bufs=3, space="PSUM") as tr_ps,
            ):
                ncopy = [0]
                pending_tr = []

                def emit_transposes():
                    for qs_t, ks_t, t in pending_tr:
                        for h in range(HPC):
                            tp = tr_ps.tile([128, 128], F16, tag="tp")
                            nc.tensor.transpose(
                                tp[:], qs_t[:, h * 128:(h + 1) * 128], ident[:])
                            dst = qt[:, h, t * 128:(t + 1) * 128]
                            if ncopy[0] % 2 == 0:
                                nc.vector.tensor_copy(dst, tp[:])
                            else:
                                nc.scalar.copy(dst, tp[:])
                            ncopy[0] += 1
                        tp = tr_ps.tile([128, 128], F16, tag="tp")
                        nc.tensor.transpose(tp[:], ks_t[:], ident[:])
                        dst = kt[:, t * 128:(t + 1) * 128]
                        if ncopy[0] % 2 == 0:
                            nc.vector.tensor_copy(dst, tp[:])
                        else:
                            nc.scalar.copy(dst, tp[:])
                        ncopy[0] += 1
                    pending_tr.clear()

                xts = [None] * KC
                for t in range(NT):             # one token tile per group
                    q_ps = qkv_ps.tile([128, QF], F32, tag="qps", name="qps")
                    kv_ps = qkv_ps.tile([128, 256], F32, tag="kvps", name="kvps")
                    for kc in range(KC):
                        if t == 0:
                            # weights ride different DMA queues than the xt
                            # stream so the JIT loads run in parallel
                            nc.scalar.dma_start(
                                wq[:, kc, :], wq_d[kc * 128:(kc + 1) * 128, :])
                            nc.gpsimd.dma_start(
                                wkv[:, kc, :], wkv_d[kc * 128:(kc + 1) * 128, :])
                        elif 1 <= t <= 4 and kc % 4 == t - 1:
                            # trickle the wo load through the QKV phase
                            nc.scalar.dma_start(
                                wo[:, kc, :], wo_d[kc * 128:(kc + 1) * 128, :])
                        if t % 2 == 0:          # 256-wide xt DMA shared by t, t+1
                            xts[kc] = xtp.tile([128, 256], F16, tag="xt",
                                               name="xt")
                            nc.sync.dma_start(
                                xts[kc][:],
                                xt_d[kc * 128:(kc + 1) * 128,
                                     (t // 2) * 256:(t // 2 + 1) * 256],
                            )
                        xt_half = xts[kc][:, (t % 2) * 128:(t % 2 + 1) * 128]
                        nc.tensor.matmul(
                            q_ps[:], xt_half, wq[:, kc, :],
                            start=(kc == 0), stop=(kc == KC - 1),
                        )
                        nc.tensor.matmul(
                            kv_ps[:], xt_half, wkv[:, kc, :],
                            start=(kc == 0), stop=(kc == KC - 1),
                        )
                    # previous tile's transposes run behind this tile's
                    # matmuls, so they never wait on the rope DVE chain
                    emit_transposes()
                    # RoPE (token-major); tables are per-head-shared with
                    # broadcast APs along the head axis
                    cq = cosk[:, t, :].unsqueeze(1).broadcast_to([128, HPC, HD])
                    sq = sink[:, t, :].rearrange("p (s x) -> p s x", s=2)
                    qp = q_ps
                    qp4 = qp[:].rearrange("p (h s x) -> p h s x", h=HPC, s=2)
                    qs_t = stage.tile([128, QF], F16, tag="qstage")
                    tmp1 = ropep.tile([128, QF], F16, tag="tmp1")
                    tmp2 = ropep.tile([128, QF], F16, tag="tmp2")
                    t24 = tmp2[:].rearrange("p (h s x) -> p h s x", h=HPC, s=2)
                    nc.vector.tensor_mul(
                        tmp1[:].rearrange("p (h f) -> p h f", h=HPC),
                        qp[:].rearrange("p (h f) -> p h f", h=HPC), cq)
                    nc.vector.tensor_mul(
                        t24[:, :, 0, :], qp4[:, :, 1, :],
                        sq[:, 0, :].unsqueeze(1).broadcast_to([128, HPC, 64]))
                    nc.vector.tensor_mul(
                        t24[:, :, 1, :], qp4[:, :, 0, :],
                        sq[:, 1, :].unsqueeze(1).broadcast_to([128, HPC, 64]))
                    nc.vector.tensor_add(qs_t[:], tmp1[:], tmp2[:])

                    kp = kv_ps
                    kp4 = kp[:, 0:HD].rearrange("p (s x) -> p s x", s=2)
                    ks_t = stage.tile([128, HD], F16, tag="kstage")
                    ktmp1 = ropep.tile([128, HD], F16, tag="ktmp1")
                    ktmp2 = ropep.tile([128, HD], F16, tag="ktmp2")
                    kt2v = ktmp2[:].rearrange("p (s x) -> p s x", s=2)
                    nc.vector.tensor_mul(ktmp1[:], kp[:, 0:HD], cosk[:, t, :])
                    nc.vector.tensor_mul(kt2v[:, 0, :], kp4[:, 1, :], sq[:, 0, :])
                    nc.vector.tensor_mul(kt2v[:, 1, :], kp4[:, 0, :], sq[:, 1, :])
                    nc.vector.tensor_add(ks_t[:], ktmp1[:], ktmp2[:])
                    nc.scalar.copy(vv[:, t, :], kp[:, HD:256])
                    pending_tr.append((qs_t, ks_t, t))
                emit_transposes()

            # ---------- phases 3-4: attention + allgather per q-chunk ----------
            with (
                tc.tile_pool(name="pt_sb", bufs=8) as ptp,
                tc.tile_pool(name="ptsum_sb", bufs=2) as ptsp,
                tc.tile_pool(name="norm_sb", bufs=2) as normp,
                tc.tile_pool(name="s_ps", bufs=3, space="PSUM") as s_ps,
                tc.tile_pool(name="acc_ps", bufs=3, space="PSUM") as acc_ps,
                tc.tile_pool(name="den_ps", bufs=1, space="PSUM") as den_ps,
                tc.tile_pool(name="bc_ps", bufs=1, space="PSUM") as bc_ps,
            ):
                den_q = []       # deferred den matmuls (stage A)
                bc_q = []        # deferred broadcast+normalize (stage B)
                bc_q_f = []      # bc closures, paired FIFO with den_q

                def emit_attention(qc):
                    nkt = 4 * qc + 4          # visible k tiles

                    for h in range(HPC):
                        ctx_ps = acc_ps.tile([128, 512], F32, tag="ctx",
                                             name="ctx")
                        state = {"ptsum": None, "pt0": None}

                        def emit_exp(ki, state=state, qc=qc, h=h):
                            d = ki - 4 * qc   # diagonal index (>=0: diagonal)
                            off = 128 * d if d > 0 else 0
                            sp = s_ps.tile([128, 512], F32, tag="sp", name="sp")
                            nc.tensor.matmul(
                                sp[:, off:512],
                                kt[:, ki * 128:(ki + 1) * 128],
                                qt[:, h, qc * 512 + off:(qc + 1) * 512],
                                start=True, stop=(d < 0),
                            )
                            if d >= 0:
                                # causal mask: add -6e4 below the diagonal via
                                # an identity matmul; exp underflows to 0
                                nc.tensor.matmul(
                                    sp[:, off:512], ident[:],
                                    dbias[:, d, off:512],
                                    start=False, stop=True,
                                )
                            pt = ptp.tile([128, 512], F16, tag="pt", name="pt")
                            nc.scalar.activation(
                                pt[:, off:512], sp[:, off:512],
                                mybir.ActivationFunctionType.Exp,
                                bias=ebias[:], scale=SCALE,
                            )
                            # incremental denominator accumulation (DVE)
                            with nc.allow_low_precision(
                                reason="denoms pre-scaled 2^-8; fp16 sum ok"
                            ):
                                if state["ptsum"] is None and state["pt0"] is None:
                                    state["pt0"] = pt
                                elif state["ptsum"] is None:
                                    ps_t = ptsp.tile([128, 512], F16,
                                                     tag="ptsum", name="ptsum")
                                    if off:
                                        # pt's [0:off] is stale; carry pt0 only
                                        nc.vector.tensor_copy(
                                            ps_t[:, 0:off],
                                            state["pt0"][:, 0:off])
                                    nc.vector.tensor_add(
                                        ps_t[:, off:512],
                                        state["pt0"][:, off:512],
                                        pt[:, off:512])
                                    state["ptsum"] = ps_t
                                else:
                                    nc.vector.tensor_add(
                                        state["ptsum"][:, off:512],
                                        state["ptsum"][:, off:512],
                                        pt[:, off:512])
                            return pt, off

                        def emit_consume(ki, pt_off, ctx_ps=ctx_ps, qc=qc,
                                         nkt=nkt):
                            pt, off = pt_off
                            nc.tensor.matmul(
                                ctx_ps[:, off:512], vv[:, ki, :], pt[:, off:512],
                                start=(ki == 0), stop=(ki == nkt - 1),
                            )

                        def make_den(state=state):
                            def emit_den():
                                den = den_ps.tile([1, 512], F32, tag="den",
                                                  name="den")
                                nc.tensor.matmul(den[:], ones_col[:],
                                                 state["ptsum"][:],
                                                 start=True, stop=True)
                                rec32 = normp.tile([1, 512], F32, tag="rec32",
                                                   name="rec32")
                                nc.vector.reciprocal_approx_fast(rec32[:],
                                                                 den[:])
                                rec16 = normp.tile([1, 512], F16, tag="rec16",
                                                   name="rec16")
                                nc.vector.tensor_copy(rec16[:], rec32[:])
                                return rec16
                            return emit_den

                        def make_bc(h=h, qc=qc, ctx_ps=ctx_ps):
                            def emit_bc(rec16):
                                bc = bc_ps.tile([128, 512], F32, tag="bc",
                                                name="bc")
                                nc.tensor.matmul(bc[:], ones_row[:], rec16[:],
                                                 start=True, stop=True)
                                bc_sb = normp.tile([128, 512], F16, tag="bc_sb",
                                                   name="bc_sb")
                                nc.scalar.copy(bc_sb[:], bc[:])
                                nc.vector.tensor_mul(
                                    ctxc[:, h, qc * 512:(qc + 1) * 512],
                                    ctx_ps[:], bc_sb[:])
                            return emit_bc

                        # software pipeline: PE runs S-matmuls ahead of the
                        # exp/mask chain; deferred norm stages slot in at
                        # fixed points so their PE ops never wait on DVE
                        DEPTH = 3
                        pend = []
                        for ki in range(nkt):
                            pend.append((ki, emit_exp(ki)))
                            if len(pend) > DEPTH:
                                emit_consume(*pend.pop(0))
                            if ki == 1 and den_q:
                                bc_q.append((den_q.pop(0)(), bc_q_f.pop(0)))
                            if ki == 3 and bc_q:
                                rec16, f = bc_q.pop(0)
                                f(rec16)
                        for item in pend:
                            emit_consume(*item)
                        den_q.append(make_den())
                        bc_q_f.append(make_bc())

                def flush_norms():
                    while den_q:
                        bc_q.append((den_q.pop(0)(), bc_q_f.pop(0)))
                    while bc_q:
                        rec16, f = bc_q.pop(0)
                        f(rec16)

                def emit_allgather(qc):
                    nc.gpsimd.dma_start(
                        cc_in[qc].rearrange("(h p) q -> p h q", p=128),
                        ctxc[:, :, qc * 512:(qc + 1) * 512],
                    )
                    nc.gpsimd.collective_compute(
                        "AllGather",
                        mybir.AluOpType.bypass,
                        replica_groups=[list(range(NC))],
                        ins=[cc_in[qc][:]],
                        outs=[cc_out[qc][:]],
                    )

                for qc in range(QCH):
                    emit_attention(qc)
                    flush_norms()
                    emit_allgather(qc)

            # ---------- phase 5: output projection ----------
            with (
                tc.tile_pool(name="ccp", bufs=16) as ccp,
                tc.tile_pool(name="osb", bufs=4) as osb,
                tc.tile_pool(name="o_ps", bufs=2, space="PSUM") as o_ps,
            ):
                for qc in range(QCH):
                    o_psum = [o_ps.tile([128, QF], F32, tag=f"ops{i}",
                                        name=f"ops{i}") for i in range(4)]
                    for fc in range(KC):
                        cc_sb = ccp.tile([128, 512], F16, tag="cc", name="cc")
                        # scalar (ACT) HWDGE queue: issues promptly, unlike the
                        # sync queue which drains attention-era bookkeeping
                        nc.scalar.dma_start(
                            cc_sb[:], cc_out[qc][fc * 128:(fc + 1) * 128, :],
                        )
                        for i in range(4):
                            nc.tensor.matmul(
                                o_psum[i][:],
                                cc_sb[:, i * 128:(i + 1) * 128],
                                wo[:, fc, :],
                                start=(fc == 0), stop=(fc == KC - 1),
                            )
                    for i in range(4):
                        t = qc * 4 + i
                        ot = osb.tile([128, QF], F32, tag="ot", name="ot")
                        if i % 2 == 0:
                            nc.vector.tensor_copy(ot[:], o_psum[i][:])
                        else:
                            nc.scalar.copy(ot[:], o_psum[i][:])
                        nc.sync.dma_start(out_d[t * 128:(t + 1) * 128, :],
                                          ot[:])

    nc.compile()
    return nc


def _prep_inputs(hidden_states, attention_mask, position_ids, wq, wk, wv, wo):
    x = np.ascontiguousarray(np.asarray(hidden_states, np.float32)[0])     # [S, D]
    pos = np.asarray(position_ids, np.int32)[0].astype(np.float32)
    wq = np.asarray(wq, np.float32)
    wk = np.asarray(wk, np.float32)
    wv = np.asarray(wv, np.float32)
    wo = np.asarray(wo, np.float32)

    f16 = np.float16
    xt = np.ascontiguousarray(x.T).astype(f16)                              # [D, S]

    freqs = 1.0 / THETA ** (np.arange(64, dtype=np.float32) / 64)
    t = pos[:, None] * freqs
    cos = np.cos(t).astype(np.float32)
    sin = np.sin(t).astype(np.float32)
    cosF = np.concatenate([cos, cos], 1)                                    # [S,128]
    sinF = np.concatenate([-sin, sin], 1)
    # [128, NT*HD]: row p, block t = table for token t*128+p
    cosk = np.ascontiguousarray(
        cosF.reshape(NT, 128, HD).transpose(1, 0, 2).reshape(128, NT * HD)
    ).astype(f16)
    sink = np.ascontiguousarray(
        sinF.reshape(NT, 128, HD).transpose(1, 0, 2).reshape(128, NT * HD)
    ).astype(f16)

    # diagonal causal bias: -6e4 where f < p + 128*r (masked), else 0;
    # added to scores pre-exp so exp underflows to exactly 0
    p = np.arange(128)[:, None, None]
    r = np.arange(QCH)[None, :, None]
    fidx = np.arange(512)[None, None, :]
    dbias = np.where(fidx >= p + 128 * r, 0.0, -6.0e4).astype(f16)

    in_maps = []
    for c in range(NC):
        in_maps.append({
            "xt": xt,
            "wq": np.ascontiguousarray(wq[:, c * QF:(c + 1) * QF]).astype(f16),
            "wkv": np.ascontiguousarray(
                np.concatenate([wk[:, c * HD:(c + 1) * HD],
                                wv[:, c * HD:(c + 1) * HD]], 1)).astype(f16),
            "wo": np.ascontiguousarray(wo[:, c * QF:(c + 1) * QF]).astype(f16),
            "cosk": cosk, "sink": sink, "dbias": dbias,
        })
    return in_maps


def run(in_maps, trace=False):
    if "nc" not in _CACHE:
        _CACHE["nc"] = build_program()
    kwargs = {}
    if trace:
        kwargs = dict(trace=True, trace_cores=list(range(NC)))
    return run_bass_kernel_spmd(_CACHE["nc"], in_maps, list(range(NC)), **kwargs)


def kernel(hidden_states, attention_mask, position_ids, wq, wk, wv, wo):
    in_maps = _prep_inputs(hidden_states, attention_mask, position_ids,
                           wq, wk, wv, wo)
    res = run(in_maps, trace=False)
    shards = [res.results[c]["out"] for c in range(NC)]
    out = np.concatenate(shards, axis=1).astype(np.float32)                 # [S, D]
    return out[None]
